# revision 1
# baseline (speedup 1.0000x reference)
"""ALiBi multi-head attention on 8 trn2 NeuronCores.

Problem: x[2, 2048, 1024], 16 heads x 64 dim, ALiBi bias -m_h*|i-j|,
softmax (non-causal), out projection. f32.

Sharding: core c = (batch b = c//4, head-group g = c%4). Each core
computes q/k/v projections for its 4 heads (columns of Wq/Wk/Wv),
per-head ALiBi attention, and a partial output projection using its
256 rows of Wo. Host sums the 4 partials per batch and adds bo.

Head-to-group assignment interleaves ALiBi slopes so that the banded
attention structure (per head-slot band limits, identical across
cores -> one SPMD graph) is load-balanced:
  group g gets heads (0-based) [15-g, 11-g, 7-g, 3-g]  (slots 0..3)

Kernel design (per core):
  - host supplies xT = x[b].T (bf16); projections are bf16 matmuls
    (fp32 PSUM accumulate; bf16 gets the fast weight-load path, f32r
    pays a serialized LDWEIGHTS per matmul). They produce Q^T,K^T
    (feature-major bf16, 2 slots packed per 128-partition slab) and
    V_aug (seq-major bf16, per-slot 65 cols: 64 v-dims + a ones
    column; V bias added as a broadcast DVE tensor_add on evacuation -
    small-K bf16 matmuls accumulating into a bf16 group give WRONG
    results on this compiler, so no K=1 bias matmul here).
  - scores computed transposed: P^T[j,i] tile = K^T_slot.T @ Q^T_slot
    (single K=64 bf16 matmul per [128,512] tile; Q pre-scaled by 1/8,
    biases folded into the PSUM evacuation tensor_scalar).
  - exp on ScalarE psum->SBUF bf16 (no max subtraction needed: raw
    scores <= ~3), then ALiBi bias applied MULTIPLICATIVELY on VectorE
    (bf16 2x mode): P = exp(s) * E, where E_s[p,v] = exp(-m_s*|v-2048-p|)
    is a host-precomputed Toeplitz master slab per slot; the bias tile
    for j-tile jt is the slice E_s[:, 2048-128*jt + i].
  - AV: O_aug^T[65, i] += V_aug_chunk.T @ P^T chunk (bf16); row 64 =
    softmax denominators for free (ones column of V_aug). AV matmuls
    are software-pipelined two j-tiles behind QK to keep the PE FIFO
    from stalling on the exp chain.
  - divide: denominator row -> SBUF -> reciprocal_approx_fast,
    broadcast via K=1 ones matmul, multiply on VectorE. Emitted
    deferred (inside the NEXT slot's loop) to hide its latency.
  - out projection: outT = Wo_g.T @ O_norm^T (f32r) accumulated over
    the two 128-row d-chunks; DMA outT [1024, 2048] to HBM.
  - banding: slot s only computes j-tiles within +-L_s of the diagonal
    (L = 28/m covers all non-negligible softmax weight; exact to ~1e-9).
    Do NOT over-tighten: sparser PE streams lose more to the HAM
    clock-gate staying cold than the skipped work saves.
  - "heater" bursts of dense full-K dummy matmuls at slot starts and
    mid-slot trip the PE HAM clock-gate back to the 2.4 GHz state;
    without them the attention phase's ~75%-duty PE stream never
    un-throttles from 1.2 GHz.
"""

import os
import sys

for _p in ("/opt/trn_rl_repo",):
    if os.path.isdir(_p) and _p not in sys.path:
        sys.path.append(_p)

import numpy as np  # noqa: E402

import concourse.bass as bass  # noqa: E402,F401
import concourse.mybir as mybir  # noqa: E402
import concourse.tile as tile  # noqa: E402
from concourse import bacc, bass_utils  # noqa: E402

F32 = mybir.dt.float32
F32R = mybir.dt.float32r
BF16 = mybir.dt.bfloat16
AF = mybir.ActivationFunctionType
ALU = mybir.AluOpType

B, S, DM, H, DH = 2, 2048, 1024, 16, 64
NCORES = 8
NJT = S // 128  # 16 j-tiles
NIB = S // 512  # 4 i-blocks

# Per-slot one-sided band limit (None = full attention). Slot s of every
# group keeps |i-j| <= L_s; L = 28/m of the widest head in the slot.
SLOT_L = [None, 1792, 448, 112]

TRACE = False
LAST_RESULT = None
_NC = None


def _slot_heads(g):
    return [15 - g, 11 - g, 7 - g, 3 - g]


def _mval(h):  # ALiBi slope for 0-based head h
    return float(2.0 ** (-8.0 * (h + 1) / 16.0))


def _band_ibs(s, jt):
    L = SLOT_L[s]
    if L is None:
        return list(range(NIB))
    lo = max(0, (128 * jt - L) // 512)
    hi = min(NIB - 1, (128 * jt + 127 + L) // 512)
    return list(range(lo, hi + 1))


def _build():
    global _NC
    if _NC is not None:
        return _NC
    nc = bacc.Bacc("TRN2", target_bir_lowering=False, debug=False,
                   num_devices=NCORES)

    def din(name, shape, dt):
        return nc.dram_tensor(name, shape, dt, kind="ExternalInput").ap()

    xT = din("xT", [DM, S], BF16)
    wq = din("wq", [DM, 256], BF16)
    wk = din("wk", [DM, 256], BF16)
    wv = din("wv", [DM, 260], BF16)
    wvb = din("wvb", [128, 260], BF16)
    wo = din("wo", [256, DM], F32R)
    ones = din("ones", [1, 128], F32R)
    eslab = din("eslab", [128, 4 * 2 * S], BF16)
    bqk = din("bqk", [128, 4], F32)
    outT = nc.dram_tensor("outT", [DM, S], F32, kind="ExternalOutput").ap()

    with tile.TileContext(nc) as tc:
        with tc.tile_pool(name="const", bufs=1) as cpool, \
             tc.tile_pool(name="slabs", bufs=1) as slabs, \
             tc.tile_pool(name="vpool", bufs=1) as vpool:

            bqk_t = cpool.tile([128, 4], F32)
            nc.sync.dma_start(bqk_t[:], bqk[:])
            ones_t = cpool.tile([1, 128], F32R)
            nc.sync.dma_start(ones_t[:], ones[:])
            wvb_t = cpool.tile([128, 260], BF16)
            nc.sync.dma_start(wvb_t[:], wvb[:])
            wo_t = cpool.tile([128, 2 * DM], F32R)
            nc.sync.dma_start(wo_t[:, 0:DM], wo[0:128, :])
            nc.sync.dma_start(wo_t[:, DM:2 * DM], wo[128:256, :])

            QS = [slabs.tile([128, S], BF16, tag=f"qs{d}", name=f"qs{d}")
                  for d in range(2)]
            KS = [slabs.tile([128, S], BF16, tag=f"ks{d}", name=f"ks{d}")
                  for d in range(2)]
            v_slab = vpool.tile([128, NJT * 260], BF16)

            # ---------------- projections ----------------
            with tc.tile_pool(name="xw", bufs=1) as xw, \
                 tc.tile_pool(name="ps1", bufs=8, space="PSUM") as ps1:
                xT_c = [xw.tile([128, S], BF16, name=f"xtc{e}")
                        for e in range(8)]
                wq_c = [xw.tile([128, 256], BF16, name=f"wqc{e}")
                        for e in range(8)]
                wk_c = [xw.tile([128, 256], BF16, name=f"wkc{e}")
                        for e in range(8)]
                wv_c = [xw.tile([128, 260], BF16, name=f"wvc{e}")
                        for e in range(8)]
                for e in range(8):
                    nc.sync.dma_start(xT_c[e][:], xT[e * 128:(e + 1) * 128, :])
                    nc.sync.dma_start(wq_c[e][:], wq[e * 128:(e + 1) * 128, :])
                    nc.sync.dma_start(wk_c[e][:], wk[e * 128:(e + 1) * 128, :])
                    nc.sync.dma_start(wv_c[e][:], wv[e * 128:(e + 1) * 128, :])

                for w_c, spair, bc0, scale in ((wq_c, QS, 0, 0.125),
                                               (wk_c, KS, 2, 1.0)):
                    pss = {(d, ib): ps1.tile([128, 512], F32, tag="p",
                                             name=f"p{d}_{ib}")
                           for d in range(2) for ib in range(NIB)}
                    for e in range(8):
                        for d in range(2):
                            for ib in range(NIB):
                                nc.tensor.matmul(
                                    pss[(d, ib)][:],
                                    w_c[e][:, d * 128:(d + 1) * 128],
                                    xT_c[e][:, ib * 512:(ib + 1) * 512],
                                    start=(e == 0), stop=(e == 7))
                    for d in range(2):
                        for ib in range(NIB):
                            # slab = psum*scale + b  (b pre-scaled on host)
                            with nc.allow_low_precision(reason="bf16 qk"):
                                nc.vector.tensor_scalar(
                                    spair[d][:, ib * 512:(ib + 1) * 512],
                                    pss[(d, ib)][:], scale,
                                    bqk_t[:, bc0 + d:bc0 + d + 1],
                                    ALU.mult, ALU.add)

                for it in range(NJT):
                    psv = ps1.tile([128, 260], F32, tag="p")
                    for e in range(8):
                        nc.tensor.matmul(
                            psv[:],
                            xT_c[e][:, it * 128:it * 128 + 128],
                            wv_c[e][:],
                            start=(e == 0), stop=(e == 7))
                    with nc.allow_low_precision(reason="bf16 v"):
                        nc.vector.tensor_add(
                            v_slab[:, it * 260:(it + 1) * 260], psv[:],
                            wvb_t[:])

            # ---------------- attention ----------------
            with tc.tile_pool(name="pt", bufs=3) as ptpool, \
                 tc.tile_pool(name="otn", bufs=1) as otnpool, \
                 tc.tile_pool(name="esl", bufs=1) as eslpool, \
                 tc.tile_pool(name="div", bufs=4) as divpool, \
                 tc.tile_pool(name="oev", bufs=2) as oevpool, \
                 tc.tile_pool(name="sc", bufs=2, space="PSUM") as scp, \
                 tc.tile_pool(name="po", bufs=4, space="PSUM") as pop:
                otn_t = [otnpool.tile([128, S], F32R, tag=f"otn{d}",
                                      name=f"otn{d}") for d in range(2)]
                # E slabs: E_s[p, v] = exp(-m_s * |v - 2048 - p|), bf16.
                # bias tile for j-tile jt is E_s[:, 2048-128*jt + i].
                es_t = eslpool.tile([128, 4 * 2 * S], BF16)
                nc.sync.dma_start(es_t[:], eslab[:])

                def emit_heater(nmm):
                    # dense full-K matmuls to trip the HAM clock-gate warm
                    hp = scp.tile([128, 512], F32, tag="sc", name="heat")
                    for i in range(nmm):
                        nc.tensor.matmul(hp[:], KS[0][:, 0:128],
                                         QS[0][:, 0:512],
                                         start=True, stop=True)

                def emit_division(s, d, po_prev):
                    for ib in range(NIB):
                        den_sb = divpool.tile([1, 512], F32, tag="den")
                        nc.vector.tensor_copy(den_sb[:],
                                              po_prev[ib][64:65, :])
                        rec_f = divpool.tile([1, 512], F32, tag="recf")
                        nc.vector.reciprocal_approx_fast(out=rec_f[:],
                                                         in_=den_sb[:])
                        rec_t = divpool.tile([1, 512], F32R, tag="rec")
                        with nc.allow_low_precision(reason="recip f32r"):
                            nc.vector.tensor_copy(rec_t[:], rec_f[:])
                        bc_ps = scp.tile([64, 512], F32, tag="sc")
                        nc.tensor.matmul(bc_ps[:], ones_t[:, 0:64], rec_t[:],
                                         start=True, stop=True)
                        bc_sb = divpool.tile([64, 512], F32, tag="bc")
                        nc.scalar.copy(bc_sb[:], bc_ps[:])
                        osl = otn_t[d][:, ib * 512:(ib + 1) * 512]
                        if s % 2 == 0:
                            nc.vector.tensor_mul(osl[0:64, :],
                                                 po_prev[ib][0:64, :],
                                                 bc_sb[:])
                        else:
                            mv_t = divpool.tile([64, 512], F32R, tag="mv")
                            nc.vector.tensor_mul(mv_t[:],
                                                 po_prev[ib][0:64, :],
                                                 bc_sb[:])
                            nc.sync.dma_start(osl[64:128, :], mv_t[:])

                pend_div = None  # (s, d, po_t) of previous slot
                for s in range(4):
                    d = s // 2
                    r0 = 64 * (s % 2)
                    q_ap = QS[d][r0:r0 + 64, :]
                    k_ap = KS[d][r0:r0 + 64, :]
                    e0 = s * 2 * S  # this slot's E slab column base
                    first_jt, last_jt = {}, {}
                    for jt in range(NJT):
                        for ib in _band_ibs(s, jt):
                            first_jt.setdefault(ib, jt)
                            last_jt[ib] = jt
                    po_t = {ib: pop.tile([65, 512], F32, tag="po",
                                         name=f"po{ib}") for ib in range(NIB)}
                    emit_heater(10)
                    pend_av = []  # software-pipeline AV two stages behind
                    for jt in range(NJT):
                        ibs = _band_ibs(s, jt)
                        pt_t = ptpool.tile([128, S], BF16, tag="pt")
                        off = 2 * S // 2 - 128 * jt  # 2048 - 128*jt (even)
                        for grp in ([i for i in ibs if i < 2],
                                    [i for i in ibs if i >= 2]):
                            if not grp:
                                continue
                            n = 512 * len(grp)
                            i0 = 512 * grp[0]
                            sc_t = scp.tile([128, 1024], F32, tag="sc")
                            for k, ib in enumerate(grp):
                                nc.tensor.matmul(
                                    sc_t[:, k * 512:(k + 1) * 512],
                                    k_ap[:, jt * 128:(jt + 1) * 128],
                                    q_ap[:, ib * 512:(ib + 1) * 512],
                                    start=True, stop=True)
                            nc.scalar.activation(pt_t[:, i0:i0 + n],
                                                 sc_t[:, 0:n], AF.Exp)
                            with nc.allow_low_precision(reason="bf16 probs"):
                                nc.vector.tensor_mul(
                                    pt_t[:, i0:i0 + n], pt_t[:, i0:i0 + n],
                                    es_t[:, e0 + off + i0:e0 + off + i0 + n])
                        if len(pend_av) == 2:
                            pjt, ppt, pibs = pend_av.pop(0)
                            for ib in pibs:
                                nc.tensor.matmul(
                                    po_t[ib][:],
                                    v_slab[:, pjt * 260 + 65 * s:
                                           pjt * 260 + 65 * s + 65],
                                    ppt[:, ib * 512:(ib + 1) * 512],
                                    start=(first_jt[ib] == pjt),
                                    stop=(last_jt[ib] == pjt))
                        pend_av.append((jt, pt_t, ibs))
                        if jt == 1 and pend_div is not None:
                            emit_division(*pend_div)
                            pend_div = None
                        if jt in (5, 10):
                            emit_heater(6)
                    for pjt, ppt, pibs in pend_av:
                        for ib in pibs:
                            nc.tensor.matmul(
                                po_t[ib][:],
                                v_slab[:, pjt * 260 + 65 * s:
                                       pjt * 260 + 65 * s + 65],
                                ppt[:, ib * 512:(ib + 1) * 512],
                                start=(first_jt[ib] == pjt),
                                stop=(last_jt[ib] == pjt))
                    pend_div = (s, d, po_t)
                emit_division(*pend_div)

                # ---------------- out projection ----------------
                emit_heater(10)
                for ct in range(8):
                    for half in range(2):
                        pp = scp.tile([128, 1024], F32, tag="sc")
                        for d in range(2):
                            for k in range(2):
                                ib = 2 * half + k
                                nc.tensor.matmul(
                                    pp[:, k * 512:(k + 1) * 512],
                                    wo_t[:, d * DM + ct * 128:
                                         d * DM + (ct + 1) * 128],
                                    otn_t[d][:, ib * 512:(ib + 1) * 512],
                                    start=(d == 0), stop=(d == 1))
                        oe_t = oevpool.tile([128, 1024], F32, tag="oe")
                        if (ct + half) % 2 == 0:
                            nc.scalar.copy(oe_t[:], pp[:])
                        else:
                            nc.vector.tensor_copy(oe_t[:], pp[:])
                        nc.sync.dma_start(
                            outT[ct * 128:(ct + 1) * 128,
                                 half * 1024:(half + 1) * 1024], oe_t[:])
    nc.compile()
    _NC = nc
    return nc


def _in_map(c, x, Wq, bq, Wk, bk, Wv, bv, Wo):
    b, g = divmod(c, 4)
    heads = _slot_heads(g)
    perm = np.concatenate([np.arange(h * DH, (h + 1) * DH) for h in heads])
    import ml_dtypes
    xTm = np.ascontiguousarray(x[b].T.astype(ml_dtypes.bfloat16))
    wq_g = np.ascontiguousarray(Wq[:, perm].astype(ml_dtypes.bfloat16))
    wk_g = np.ascontiguousarray(Wk[:, perm].astype(ml_dtypes.bfloat16))
    wv_g = np.zeros((DM, 260), np.float32)
    wvb_r = np.zeros((260,), np.float32)
    for s, h in enumerate(heads):
        wv_g[:, 65 * s:65 * s + 64] = Wv[:, h * DH:(h + 1) * DH]
        wvb_r[65 * s:65 * s + 64] = bv[h * DH:(h + 1) * DH]
        wvb_r[65 * s + 64] = 1.0

    wv_g = wv_g.astype(ml_dtypes.bfloat16)
    wvb_r = np.ascontiguousarray(np.tile(wvb_r[None, :], (128, 1)).astype(ml_dtypes.bfloat16))
    wo_g = np.ascontiguousarray(Wo[perm, :])
    bq_g = bq[perm] * 0.125  # q scale folded into evac: (ps + b)*0.125
    bk_g = bk[perm]
    bqk_m = np.stack([bq_g[0:128], bq_g[128:256],
                      bk_g[0:128], bk_g[128:256]], axis=1)
    bqk_m = np.ascontiguousarray(bqk_m.astype(np.float32))
    ones_m = np.ones((1, 128), np.float32)
    vv = np.arange(2 * S, dtype=np.float32)
    dist = np.abs(vv[None, :] - S - np.arange(128, dtype=np.float32)[:, None])
    import ml_dtypes
    esl = np.zeros((128, 4 * 2 * S), np.float32)
    for sl in range(4):
        esl[:, sl * 2 * S:(sl + 1) * 2 * S] = np.exp(-_mval(heads[sl]) * dist)
    esl_m = np.ascontiguousarray(esl.astype(ml_dtypes.bfloat16))
    return {"xT": xTm, "eslab": esl_m, "wq": wq_g, "wk": wk_g, "wv": wv_g, "wvb": wvb_r,
            "wo": wo_g, "ones": ones_m, "bqk": bqk_m}


def kernel(**inputs):
    global LAST_RESULT
    x = np.asarray(inputs["x"], np.float32)
    Wq = np.asarray(inputs["Wq"], np.float32)
    bq = np.asarray(inputs["bq"], np.float32)
    Wk = np.asarray(inputs["Wk"], np.float32)
    bk = np.asarray(inputs["bk"], np.float32)
    Wv = np.asarray(inputs["Wv"], np.float32)
    bv = np.asarray(inputs["bv"], np.float32)
    Wo = np.asarray(inputs["Wo"], np.float32)
    bo = np.asarray(inputs["bo"], np.float32)

    nc = _build()
    in_maps = [_in_map(c, x, Wq, bq, Wk, bk, Wv, bv, Wo)
               for c in range(NCORES)]
    res = bass_utils.run_bass_kernel_spmd(nc, in_maps,
                                          core_ids=list(range(NCORES)),
                                          trace=TRACE)
    LAST_RESULT = res
    out = np.zeros((B, S, DM), np.float32)
    for c in range(NCORES):
        out[c // 4] += res.results[c]["outT"].T
    out += bo[None, None, :]
    return out



# revision 7
# speedup vs baseline: 3.1592x; 3.1592x over previous
"""ALiBi multi-head attention on 8 trn2 NeuronCores.

Problem: x[2, 2048, 1024], 16 heads x 64 dim, ALiBi bias -m_h*|i-j|,
softmax (non-causal), out projection. f32.

Sharding: core c = (batch b = c//4, head-group g = c%4). Each core
computes q/k/v projections for its 4 heads (columns of Wq/Wk/Wv),
per-head ALiBi attention, and a partial output projection using its
256 rows of Wo. Host sums the 4 partials per batch and adds
bo + bv@Wo (the V bias commutes through softmax averaging, so it is
folded into the host-side output bias).

Head-to-group assignment interleaves ALiBi slopes (slot s of group g
gets head 15-g-4s) so per-slot band limits are identical across cores
(one SPMD graph).

Kernel design (per core), v2 ("folded-bias"):
  - projections as bf16 matmuls (f32 PSUM). Q/K evacuate via ScalarE
    activation Identity (scale+per-partition bias) into FOUR per-slot
    slabs [68, 2048]: rows 0-63 = head features, rows 64-67 = ALiBi
    augmentation rows (host constants). V evacuates via ScalarE Copy
    to a bf16 slab; the softmax-denominator "ones" columns are DMA'd
    in with a strided access pattern.
  - ALiBi bias is folded INTO the QK matmul as a rank-2 term over 4
    extra contraction rows (bf16 hi/lo residual pairs for precision):
    K-side aug [mj_hi, mj_lo, 1, 1] (and a negated copy for
    below-diagonal tiles, filled by SBUF->SBUF DMA), Q-side aug
    [1, 1, -mi_hi, -mi_lo]. A K=68 matmul then yields
    q.k/8 - m|i-j| directly in PSUM for any tile strictly above or
    below the diagonal -- no elementwise bias work at all.
  - only the 128x128 true-diagonal block per j-tile needs elementwise
    bias: probs = exp(score) * E where E[p,c] = exp(-m|c-p|) is a
    fixed per-slot [128,128] constant; that multiply runs on GpSimd
    (Pool), which is otherwise idle.
  - exp: ScalarE activation Exp for slots 1-3; slot 0 (half the
    elements) is split between ScalarE and a DVE "Schraudolph" exp
    (one tensor_scalar: i16 = round(s*128/ln2 + 16251), buffer
    bitcast to bf16), balancing the two engines. Slot-0 heads are the
    flattest (huge softmax support), so the ~1.5% exp noise averages
    out far below the error budget.
  - bands tightened to L ~= 9.5/m (tail mass e^-9.5, invisible at
    2e-2 tolerance): slots 0,1 at 512-col granularity, slots 2,3 at
    128-col granularity.
  - AV: O_aug^T[65, i] += V_aug.T @ P^T per block (denominator row
    free via the ones column). Software-pipelined 2 j-tiles behind QK.
  - division per 512-i-block, emitted as soon as that block's last AV
    lands: den row -> ScalarE copy -> DVE reciprocal_approx_fast ->
    ScalarE Copy cast to f32r -> ones-matmul broadcast -> ScalarE
    evac -> DVE multiply into otn (f32r). Odd slots route through a
    staging tile + DMA (engine partition-shift-up restriction).
  - out projection: outT = Wo_g.T @ O_norm^T (f32r, N=512 so full
    rate), evacuated to bf16 (halves the output DMA; host upcasts).
  - no heaters: PE duty is high enough that the HAM clock-gate stays
    warm on its own; total PE rows ~275k vs ~378k for v1.
"""

import os
import sys

for _p in ("/opt/trn_rl_repo",):
    if os.path.isdir(_p) and _p not in sys.path:
        sys.path.append(_p)

import numpy as np  # noqa: E402

import concourse.bass as bass  # noqa: E402,F401
import concourse.mybir as mybir  # noqa: E402
import concourse.tile as tile  # noqa: E402
from concourse import bacc, bass_utils  # noqa: E402

F32 = mybir.dt.float32
F32R = mybir.dt.float32r
BF16 = mybir.dt.bfloat16
I16 = mybir.dt.int16
AF = mybir.ActivationFunctionType
ALU = mybir.AluOpType

B, S, DM, H, DH = 2, 2048, 1024, 16, 64
NCORES = 8
NJT = S // 128  # 16 j-tiles

# Per-slot one-sided band limit (None = full) and i-block granularity.
SLOT_L = [None, 608, 152, 40]
SLOT_GR = [512, 512, 128, 128]
# slot-0 j-tiles whose exp runs on DVE (Schraudolph); rest on ScalarE
S0_DVE = frozenset(int(x) for x in os.environ.get("S0_DVE", "0,1,2,4,5,6,8,9,10,12,13,14").split(",") if x != "")
_PHASE = int(os.environ.get("KPHASE", "3"))
SCHRAUD_A = 128.0 / float(np.log(2.0))
SCHRAUD_B = 16251.0

TRACE = False
LAST_RESULT = None
_NC = None


def _slot_heads(g):
    return [15 - g, 11 - g, 7 - g, 3 - g]


def _mval(h):  # ALiBi slope for 0-based head h
    return float(2.0 ** (-8.0 * (h + 1) / 16.0))


def _blocks(s, jt):
    gr, L = SLOT_GR[s], SLOT_L[s]
    nb = S // gr
    if L is None:
        return 0, nb - 1
    return max(0, (128 * jt - L) // gr), min(nb - 1, (128 * jt + 127 + L) // gr)


def _build():
    global _NC
    if _NC is not None:
        return _NC
    nc = bacc.Bacc("TRN2", target_bir_lowering=False, debug=False,
                   num_devices=NCORES)

    def din(name, shape, dt):
        return nc.dram_tensor(name, shape, dt, kind="ExternalInput").ap()

    xT = din("xT", [DM, S], BF16)
    wq = din("wq", [DM, 256], BF16)
    wk = din("wk", [DM, 256], BF16)
    wv = din("wv", [DM, 260], BF16)
    wo = din("wo", [256, DM], F32R)
    onesr = din("onesr", [1, 128], F32R)
    bqk = din("bqk", [128, 4], F32)
    qaug = din("qaug", [16, S], BF16)
    kaugp = din("kaugp", [16, S], BF16)
    kaugm = din("kaugm", [16, S], BF16)
    ediag = din("ediag", [128, 512], BF16)
    vones = din("vones", [128, 64], BF16)
    outT = nc.dram_tensor("outT", [DM, S], BF16, kind="ExternalOutput").ap()

    with tile.TileContext(nc) as tc:
        with tc.tile_pool(name="const", bufs=1) as cpool, \
             tc.tile_pool(name="slabs", bufs=1) as slabs, \
             tc.tile_pool(name="vpool", bufs=1) as vpool:

            bqk_t = cpool.tile([128, 4], F32)
            nc.sync.dma_start(bqk_t[:], bqk[:])
            ones_t = cpool.tile([1, 128], F32R)
            nc.sync.dma_start(ones_t[:], onesr[:])
            ediag_t = cpool.tile([128, 512], BF16)
            nc.sync.dma_start(ediag_t[:], ediag[:])
            wo_t = cpool.tile([128, 2 * DM], F32R)
            nc.sync.dma_start(wo_t[:, 0:DM], wo[0:128, :])
            nc.sync.dma_start(wo_t[:, DM:2 * DM], wo[128:256, :])

            QS = [slabs.tile([68, S], BF16, name=f"qs{s}") for s in range(4)]
            KP = [slabs.tile([68, S], BF16, name=f"kp{s}") for s in range(4)]
            KM = [slabs.tile([68, S], BF16, name=f"km{s}") for s in range(4)]
            v_slab = vpool.tile([128, NJT * 260], BF16)

            for s in range(4):
                nc.sync.dma_start(QS[s][64:68, :], qaug[4 * s:4 * s + 4, :])
                nc.sync.dma_start(KP[s][64:68, :], kaugp[4 * s:4 * s + 4, :])
                nc.sync.dma_start(KM[s][64:68, :], kaugm[4 * s:4 * s + 4, :])

            # ---------------- projections ----------------
            with tc.tile_pool(name="xw", bufs=1) as xw, \
                 tc.tile_pool(name="ps1", bufs=8, space="PSUM") as ps1:
                xT_c = [xw.tile([128, S], BF16, name=f"xtc{e}")
                        for e in range(8)]
                wq_c = [xw.tile([128, 256], BF16, name=f"wqc{e}")
                        for e in range(8)]
                wk_c = [xw.tile([128, 256], BF16, name=f"wkc{e}")
                        for e in range(8)]
                wv_c = [xw.tile([128, 260], BF16, name=f"wvc{e}")
                        for e in range(8)]
                for e in range(8):
                    nc.sync.dma_start(xT_c[e][:], xT[e * 128:(e + 1) * 128, :])
                    nc.sync.dma_start(wq_c[e][:], wq[e * 128:(e + 1) * 128, :])
                    nc.sync.dma_start(wk_c[e][:], wk[e * 128:(e + 1) * 128, :])
                    nc.sync.dma_start(wv_c[e][:], wv[e * 128:(e + 1) * 128, :])

                for w_c, dst, bc0, scale in ((wq_c, QS, 0, 0.125),
                                             (wk_c, KP, 2, 1.0)):
                    pss = {(d, ib): ps1.tile([128, 512], F32, tag="p",
                                             name=f"p{d}_{ib}")
                           for d in range(2) for ib in range(4)}
                    for e in range(8):
                        for d in range(2):
                            for ib in range(4):
                                nc.tensor.matmul(
                                    pss[(d, ib)][:],
                                    w_c[e][:, d * 128:(d + 1) * 128],
                                    xT_c[e][:, ib * 512:(ib + 1) * 512],
                                    start=(e == 0), stop=(e == 7))
                    for d in range(2):
                        for ib in range(4):
                            cols = slice(ib * 512, (ib + 1) * 512)
                            with nc.allow_low_precision(reason="bf16 qk"):
                                nc.scalar.activation(
                                    dst[2 * d][0:64, cols],
                                    pss[(d, ib)][0:64, :], AF.Identity,
                                    bias=bqk_t[0:64, bc0 + d:bc0 + d + 1],
                                    scale=scale)
                                nc.scalar.activation(
                                    dst[2 * d + 1][0:64, cols],
                                    pss[(d, ib)][64:128, :], AF.Identity,
                                    bias=bqk_t[64:128, bc0 + d:bc0 + d + 1],
                                    scale=scale)

                for s in range(4):
                    nc.sync.dma_start(KM[s][0:64, :], KP[s][0:64, :])

                for it in range(NJT):
                    psv = ps1.tile([128, 260], F32, tag="p")
                    for e in range(8):
                        nc.tensor.matmul(
                            psv[:],
                            xT_c[e][:, it * 128:it * 128 + 128],
                            wv_c[e][:],
                            start=(e == 0), stop=(e == 7))
                    with nc.allow_low_precision(reason="bf16 v"):
                        nc.scalar.activation(
                            v_slab[:, it * 260:(it + 1) * 260], psv[:],
                            AF.Copy)
                nc.sync.dma_start(v_slab[:, 64:NJT * 260:65], vones[:])

            # ---------------- attention ----------------
            with tc.tile_pool(name="pt", bufs=4) as ptpool, \
                 tc.tile_pool(name="otn", bufs=1) as otnpool, \
                 tc.tile_pool(name="div", bufs=4) as divpool, \
                 tc.tile_pool(name="oev", bufs=2) as oevpool, \
                 tc.tile_pool(name="sc", bufs=2, space="PSUM") as scp, \
                 tc.tile_pool(name="po", bufs=4, space="PSUM") as pop:
                otn_t = [otnpool.tile([128, S], F32R, name=f"otn{d}")
                         for d in range(2)]

                def emit_division(s, ib, po_t):
                    d = s // 2
                    den = divpool.tile([1, 512], F32, tag="den")
                    nc.scalar.copy(den[:], po_t[ib][64:65, :])
                    rec_f = divpool.tile([1, 512], F32, tag="recf")
                    nc.vector.reciprocal_approx_fast(out=rec_f[:],
                                                     in_=den[:])
                    rec_t = divpool.tile([1, 512], F32R, tag="rec")
                    with nc.allow_low_precision(reason="recip f32r"):
                        nc.scalar.activation(rec_t[:], rec_f[:], AF.Copy)
                    bc_ps = scp.tile([64, 512], F32, tag="sc")
                    nc.tensor.matmul(bc_ps[:], ones_t[:, 0:64], rec_t[:],
                                     start=True, stop=True)
                    bc_sb = divpool.tile([64, 512], F32, tag="bc")
                    nc.scalar.copy(bc_sb[:], bc_ps[:])
                    osl = otn_t[d][:, ib * 512:(ib + 1) * 512]
                    with nc.allow_low_precision(reason="otn f32r"):
                        if s % 2 == 0:
                            nc.vector.tensor_mul(osl[0:64, :],
                                                 po_t[ib][0:64, :],
                                                 bc_sb[:])
                        else:
                            mv_t = divpool.tile([64, 512], F32R, tag="mv")
                            nc.vector.tensor_mul(mv_t[:],
                                                 po_t[ib][0:64, :],
                                                 bc_sb[:])
                            nc.sync.dma_start(osl[64:128, :], mv_t[:])

                div_queue = []  # deferred (s, ib, po_t) emissions

                def drain_div(n):
                    for _ in range(min(n, len(div_queue))):
                        emit_division(*div_queue.pop(0))

                for s in range(4 if _PHASE >= 2 else 0):
                    gr = SLOT_GR[s]
                    ecol = 128 * s  # ediag col base
                    # PSUM accumulation groups are zero-region (bank)
                    # granular: start/stop per 512-col po region, not per
                    # gr-block.
                    reg_first, reg_last = {}, {}
                    for jt in range(NJT):
                        lo, hi = _blocks(s, jt)
                        for bb in range(lo, hi + 1):
                            ib = bb * gr // 512
                            reg_first.setdefault(ib, jt)
                            reg_last[ib] = jt
                    bpr = 512 // gr  # blocks per po region
                    po_t = {ib: pop.tile([65, 512], F32, tag="po",
                                         name=f"po{ib}") for ib in range(4)}

                    def emit_av(pjt, ppt, plo, phi):
                        for bb in range(plo, phi + 1):
                            ib, c0 = bb * gr // 512, (bb * gr) % 512
                            st = (reg_first[ib] == pjt
                                  and bb == max(plo, ib * bpr))
                            sp = (reg_last[ib] == pjt
                                  and bb == min(phi, (ib + 1) * bpr - 1))
                            nc.tensor.matmul(
                                po_t[ib][:, c0:c0 + gr],
                                v_slab[:, pjt * 260 + 65 * s:
                                       pjt * 260 + 65 * s + 65],
                                ppt[:, bb * gr:(bb + 1) * gr],
                                start=st, stop=sp)
                        for ib in range(4):
                            if reg_last.get(ib) == pjt:
                                div_queue.append((s, ib, po_t))

                    pend_av = []  # software-pipeline AV two j-tiles behind
                    for jt in range(NJT):
                        lo, hi = _blocks(s, jt)
                        db = (128 * jt) // gr
                        r = 128 * jt - gr * db
                        pt_t = ptpool.tile([128, S], BF16, tag="pt")
                        # group blocks into <=1024-col psum tiles
                        groups = []
                        if gr == 512:
                            for b0 in range(lo, hi + 1, 2):
                                groups.append((b0, min(b0 + 1, hi)))
                        else:
                            groups.append((lo, hi))
                        for g0, g1 in groups:
                            w = (g1 - g0 + 1) * gr
                            i0 = g0 * gr
                            sc_t = scp.tile([128, 1024], F32, tag="sc")
                            # a start=True matmul zeroes its whole 2KB psum
                            # bank: all matmuls sharing a bank form ONE
                            # accumulation group (start on first write,
                            # stop on last).
                            pieces = []  # (off, width, kslab, krows)
                            for bb in range(g0, g1 + 1):
                                off = (bb - g0) * gr
                                if bb != db:
                                    kside = KP[s] if bb > db else KM[s]
                                    pieces.append((off, gr, kside, 68))
                                else:
                                    if r > 0:
                                        pieces.append((off, r, KM[s], 68))
                                    pieces.append((off + r, 128, KP[s], 64))
                                    if r + 128 < gr:
                                        pieces.append((off + r + 128,
                                                       gr - r - 128,
                                                       KP[s], 68))
                            nbank = {}
                            for off, w_, _, _ in pieces:
                                nbank[off // 512] = nbank.get(off // 512,
                                                              0) + 1
                            seen = {}
                            jc = slice(jt * 128, (jt + 1) * 128)
                            for off, w_, kslab, krows in pieces:
                                bk = off // 512
                                seen[bk] = seen.get(bk, 0) + 1
                                icol = i0 + off
                                nc.tensor.matmul(
                                    sc_t[:, off:off + w_],
                                    kslab[0:krows, jc],
                                    QS[s][0:krows, icol:icol + w_],
                                    start=(seen[bk] == 1),
                                    stop=(seen[bk] == nbank[bk]))
                            with nc.allow_low_precision(reason="bf16 probs"):
                                if s == 0 and jt in S0_DVE:
                                    nc.vector.tensor_scalar(
                                        pt_t[:, i0:i0 + w].bitcast(I16),
                                        sc_t[:, 0:w], SCHRAUD_A, SCHRAUD_B,
                                        ALU.mult, ALU.add)
                                else:
                                    nc.scalar.activation(
                                        pt_t[:, i0:i0 + w], sc_t[:, 0:w],
                                        AF.Exp)
                        # diagonal 128-col block: multiply by E
                        # (DVE; TensorTensor on GpSimd crashes the HW
                        # exec unit on this runtime)
                        dcols = slice(128 * jt, 128 * jt + 128)
                        with nc.allow_low_precision(reason="bf16 probs"):
                            nc.vector.tensor_mul(
                                pt_t[:, dcols], pt_t[:, dcols],
                                ediag_t[:, ecol:ecol + 128])
                        if len(pend_av) == 2:
                            emit_av(*pend_av.pop(0))
                        pend_av.append((jt, pt_t, lo, hi))
                        drain_div(1)
                    for pa in pend_av:
                        emit_av(*pa)
                        drain_div(1)
                drain_div(99)

                # ---------------- out projection ----------------
                for ct in range(8 if _PHASE >= 3 else 0):
                    for half in range(2):
                        pp = scp.tile([128, 1024], F32, tag="sc")
                        for d in range(2):
                            for k in range(2):
                                ib = 2 * half + k
                                nc.tensor.matmul(
                                    pp[:, k * 512:(k + 1) * 512],
                                    wo_t[:, d * DM + ct * 128:
                                         d * DM + (ct + 1) * 128],
                                    otn_t[d][:, ib * 512:(ib + 1) * 512],
                                    start=(d == 0), stop=(d == 1))
                        oe_t = oevpool.tile([128, 1024], BF16, tag="oe")
                        with nc.allow_low_precision(reason="bf16 out"):
                            if (ct + half) % 2 == 0:
                                nc.scalar.copy(oe_t[:], pp[:])
                            else:
                                nc.vector.tensor_copy(oe_t[:], pp[:])
                        nc.sync.dma_start(
                            outT[ct * 128:(ct + 1) * 128,
                                 half * 1024:(half + 1) * 1024], oe_t[:])
    nc.compile()
    _NC = nc
    return nc


def _in_map(c, x, Wq, bq, Wk, bk, Wv, Wo):
    import ml_dtypes
    b, g = divmod(c, 4)
    heads = _slot_heads(g)
    perm = np.concatenate([np.arange(h * DH, (h + 1) * DH) for h in heads])
    xTm = np.ascontiguousarray(x[b].T.astype(ml_dtypes.bfloat16))
    wq_g = np.ascontiguousarray(Wq[:, perm].astype(ml_dtypes.bfloat16))
    wk_g = np.ascontiguousarray(Wk[:, perm].astype(ml_dtypes.bfloat16))
    wv_g = np.zeros((DM, 260), np.float32)
    for s, h in enumerate(heads):
        wv_g[:, 65 * s:65 * s + 64] = Wv[:, h * DH:(h + 1) * DH]
    wv_g = wv_g.astype(ml_dtypes.bfloat16)
    wo_g = np.ascontiguousarray(Wo[perm, :])
    bq_g = bq[perm] * 0.125  # q scale folded into evac: ps*0.125 + b*0.125
    bk_g = bk[perm]
    bqk_m = np.stack([bq_g[0:128], bq_g[128:256],
                      bk_g[0:128], bk_g[128:256]], axis=1)
    bqk_m = np.ascontiguousarray(bqk_m.astype(np.float32))
    ones_m = np.ones((1, 128), np.float32)

    pos = np.arange(S, dtype=np.float64)
    qaug = np.zeros((16, S), np.float32)
    kaugp = np.zeros((16, S), np.float32)
    ediag = np.zeros((128, 512), np.float32)
    dd = np.abs(np.arange(128)[:, None] - np.arange(128)[None, :])
    for s, h in enumerate(heads):
        m = _mval(h)
        mi = m * pos
        hi = mi.astype(ml_dtypes.bfloat16).astype(np.float64)
        lo = (mi - hi).astype(ml_dtypes.bfloat16).astype(np.float64)
        qaug[4 * s + 0] = 1.0
        qaug[4 * s + 1] = 1.0
        qaug[4 * s + 2] = -hi
        qaug[4 * s + 3] = -lo
        kaugp[4 * s + 0] = hi
        kaugp[4 * s + 1] = lo
        kaugp[4 * s + 2] = 1.0
        kaugp[4 * s + 3] = 1.0
        ediag[:, 128 * s:128 * s + 128] = np.exp(-m * dd)
    qaug_m = np.ascontiguousarray(qaug.astype(ml_dtypes.bfloat16))
    kaugp_m = np.ascontiguousarray(kaugp.astype(ml_dtypes.bfloat16))
    kaugm_m = np.ascontiguousarray((-kaugp).astype(ml_dtypes.bfloat16))
    ediag_m = np.ascontiguousarray(ediag.astype(ml_dtypes.bfloat16))
    vones_m = np.ones((128, 64), ml_dtypes.bfloat16)
    return {"xT": xTm, "wq": wq_g, "wk": wk_g, "wv": wv_g, "wo": wo_g,
            "onesr": ones_m, "bqk": bqk_m, "qaug": qaug_m,
            "kaugp": kaugp_m, "kaugm": kaugm_m, "ediag": ediag_m,
            "vones": vones_m}


def kernel(**inputs):
    global LAST_RESULT
    x = np.asarray(inputs["x"], np.float32)
    Wq = np.asarray(inputs["Wq"], np.float32)
    bq = np.asarray(inputs["bq"], np.float32)
    Wk = np.asarray(inputs["Wk"], np.float32)
    bk = np.asarray(inputs["bk"], np.float32)
    Wv = np.asarray(inputs["Wv"], np.float32)
    bv = np.asarray(inputs["bv"], np.float32)
    Wo = np.asarray(inputs["Wo"], np.float32)
    bo = np.asarray(inputs["bo"], np.float32)

    nc = _build()
    in_maps = [_in_map(c, x, Wq, bq, Wk, bk, Wv, Wo)
               for c in range(NCORES)]
    res = bass_utils.run_bass_kernel_spmd(nc, in_maps,
                                          core_ids=list(range(NCORES)),
                                          trace=TRACE)
    LAST_RESULT = res
    out = np.zeros((B, S, DM), np.float32)
    for c in range(NCORES):
        out[c // 4] += res.results[c]["outT"].T.astype(np.float32)
    out += (bo + bv @ Wo)[None, None, :]
    return out
